# revision 41
# baseline (speedup 1.0000x reference)
"""Trainium2 Bass kernel for nn_NodeConv (GNN message passing).

Strategy (8 NeuronCores, data-parallel, no collectives):
  - Nodes are partitioned into 8 contiguous ranges; every edge is routed to
    the core that owns its *destination* node, so the segment-sum is fully
    local to each core.  MLP weights are replicated.
  - On the host, each core's nodes are sorted by in-degree and packed into
    supergroups of 512.  Edge features are laid out FEATURE-MAJOR in a
    prefix-ELL slab: for supergroup s and edge-slot k, one column per node
    that still has a k-th edge (a prefix of the degree-sorted nodes, so
    padding is ~0.6%).  The supergroup's 512 xT columns ride at the END of
    its slab block, so x needs no separate DMA stream and arrives exactly
    when the MLP needs it.
  - On the device the segment-sum runs on the TensorEngine as accumulating
    pass-through matmuls: matmul(lhsT=I, rhs=slot_block) adds each slot's
    [128 feat x <=512 node] block into the supergroup's PSUM message tile.
  - The MLP runs feature-major (weights as lhsT, 512-node batches); the
    last layer uses the activations as lhsT which transposes the result to
    node-major for GroupNorm.  The residual x is recovered node-major by
    4 PE transposes of the slab's xT block (no second x copy from HBM), and
    the output is written back as fp16 (~58 MB/core total HBM traffic).
  - A warm-up burst of dummy matmuls at kernel start lifts the PE_HAM clock
    gate (1.2 -> 2.4 GHz) before the first real matmul, so the PE never
    lags the DMA stream and the end-of-stream drain stays short.
"""

import sys

sys.path.insert(0, "/opt/trn_rl_repo")

import numpy as np
import ml_dtypes

import concourse.bass as bass
import concourse.bacc as bacc
import concourse.tile as tile
from concourse import mybir
from concourse.bass_utils import run_bass_kernel_spmd

# bass_utils imports antenv.axon_hooks unconditionally when tracing is
# requested; the image's antenv lacks that module.  Provide a null registry
# so a BASS_TRACE env var can't crash the run.
try:
    import antenv.axon_hooks  # noqa: F401
except ImportError:
    import types as _types
    import antenv as _antenv
    _m = _types.ModuleType("antenv.axon_hooks")
    _m._hook = None
    _m.set_axon_ntff_profile_hook = lambda h, _m=_m: setattr(_m, "_hook", h)
    _m.get_axon_ntff_profile_hook = lambda _m=_m: _m._hook
    sys.modules["antenv.axon_hooks"] = _m
    _antenv.axon_hooks = _m

P = 128
N_CORES = 8
SGN = 512       # nodes per supergroup (MLP batch)
NG = SGN // P   # 128-node groups per supergroup
OUT_BATCH = 5   # supergroups per output DMA (bulk; the tail writes smaller)
CHUNK = 2048    # slab DMA chunk size in columns (~524 KB)
N_WARM = 36     # PE warm-up matmuls (~3.9 us at the cold 1.2 GHz clock)
TAPER_W = 192   # slots narrower than this are pre-combined on GpSimd
EPS = 1e-5

F8 = mybir.dt.float8e4
F16 = mybir.dt.float16
F32 = mybir.dt.float32
AF = mybir.ActivationFunctionType
ALU = mybir.AluOpType


# --------------------------------------------------------------------------
# Host-side sharding / layout
# --------------------------------------------------------------------------

def _host_prep(x, e, edge_index):
    """Shard nodes/edges across cores and build per-core feature-major
    prefix-ELL slabs with the xT block appended per supergroup."""
    n_nodes = x.shape[0]
    npc = -(-n_nodes // N_CORES)              # nodes per core (ceil)
    dst = np.asarray(edge_index[1]).astype(np.int64)
    e16 = np.ascontiguousarray(e, dtype=np.float16)
    e16z = np.vstack([e16, np.zeros((1, e16.shape[1]), np.float16)])
    zero_row = e16.shape[0]

    nsg = -(-npc // SGN)
    npc_pad = nsg * SGN

    cores = []
    for c in range(N_CORES):
        lo, hi = c * npc, min((c + 1) * npc, n_nodes)
        sel = np.nonzero((dst >= lo) & (dst < hi))[0]
        ldst = (dst[sel] - lo).astype(np.int64)
        n_real = hi - lo
        deg = np.bincount(ldst, minlength=npc)
        degp = np.zeros(npc_pad, np.int64)
        degp[:npc] = deg
        order = np.argsort(-degp, kind="stable")      # padded ids, deg desc
        order_e = np.argsort(ldst, kind="stable")
        esort = sel[order_e]                          # edges grouped by dst
        starts = np.zeros(npc_pad + 1, np.int64)
        np.cumsum(degp, out=starts[1:])
        cores.append(dict(lo=lo, n_real=n_real, degp=degp, order=order,
                          esort=esort, starts=starts))

    # canonical per-(supergroup, slot) active-prefix schedule: shared by all
    # cores (max over cores), slot 0 always covers the full 512 columns so
    # the first accumulating matmul initializes the whole PSUM bank.
    d_sg = np.ones(nsg, np.int64)
    for c in cores:
        sdeg = c["degp"][c["order"]].reshape(nsg, SGN)
        d_sg = np.maximum(d_sg, sdeg.max(axis=1))
    nact = []
    for s in range(nsg):
        na = np.zeros(int(d_sg[s]), np.int64)
        for c in cores:
            sdeg = c["degp"][c["order"]][s * SGN:(s + 1) * SGN]
            for k in range(int(d_sg[s])):
                na[k] = max(na[k], int((sdeg > k).sum()))
        na[0] = SGN
        nact.append(na)

    # processing order: LIGHT-to-HEAVY.  Light supergroups are PE-bound
    # (the fixed MLP/GN cost dominates their few message columns) while
    # heavy ones are DMA-bound, so ending with the heavy supergroups lets
    # the PE catch up during the final deliveries instead of draining a
    # backlog after the stream ends.  The single lightest supergroup goes
    # very last: its short compute chain minimizes the post-stream drain.
    proc = list(range(nsg - 2, -1, -1)) + [nsg - 1]

    # per-supergroup slab block = [slot columns][512 xT columns]
    coff = np.zeros(nsg + 1, np.int64)
    mcols = np.zeros(nsg, np.int64)          # message-slot columns per block
    for b, s in enumerate(proc):
        mcols[b] = int(nact[s].sum())
        coff[b + 1] = coff[b] + mcols[b] + SGN
    C = int(coff[-1])

    # slot-aligned slab DMA chunks of ~CHUNK columns; the first supergroups
    # use smaller chunks so the PE's chunk-waits during the DMA ramp stay
    # well under the ~3.4us HAM idle window (a longer wait re-throttles the
    # PE clock to 1.2 GHz).  The xT block merges into the last chunk.
    split = []
    for i, s in enumerate(proc):
        cum = np.cumsum(nact[s])
        steps = iter([512, 1024] if i == 0 else [])
        thr = next(steps, CHUNK)
        bounds = [0]
        for k in range(len(cum)):
            if int(cum[k]) - bounds[-1] >= thr and k + 1 < len(cum):
                bounds.append(int(cum[k]))
                thr = next(steps, CHUNK)
        bounds.append(int(mcols[i]) + SGN)
        split.append(bounds)

    in_maps = []
    for c in cores:
        order, degp, starts, esort = \
            c["order"], c["degp"], c["starts"], c["esort"]
        # xT feature-major, supergroups in processing order
        xp = np.zeros((npc_pad, P), np.float32)
        xr = np.asarray(x[c["lo"]:c["lo"] + c["n_real"]], np.float32)
        valid = order < c["n_real"]
        xp[np.nonzero(valid)[0]] = xr[order[valid]]
        xsg = xp.reshape(nsg, SGN, P)[proc].astype(np.float16)  # [nsg,512,128]

        slab = np.empty(C * P, np.float16)
        for b, s in enumerate(proc):
            mc = int(mcols[b])
            idx = np.full(mc, zero_row, np.int64)
            pos = 0
            for k in range(int(d_sg[s])):
                n = int(nact[s][k])
                nid = order[s * SGN:s * SGN + n]
                valid_k = degp[nid] > k
                p = starts[nid] + k
                idx[pos:pos + n] = np.where(
                    valid_k, esort[np.minimum(p, len(esort) - 1)], zero_row)
                pos += n
            # block = [mc slot cols][512 xT cols], feature-major
            arr = np.concatenate([e16z[idx], xsg[b]], axis=0)
            c0 = int(coff[b])
            for j in range(len(split[b]) - 1):
                lo2, hi2 = split[b][j], split[b][j + 1]
                slab[(c0 + lo2) * P:(c0 + hi2) * P] = \
                    arr[lo2:hi2].T.reshape(-1)
        in_maps.append(dict(e_slab=slab))

    meta = dict(npc=npc, nsg=nsg, npc_pad=npc_pad, d_sg=d_sg, nact=nact,
                proc=proc, coff=coff, mcols=mcols, C=C, split=split,
                cores=cores)
    return in_maps, meta


# --------------------------------------------------------------------------
# Device program
# --------------------------------------------------------------------------

def _build_program(meta, flags):
    nsg, npc_pad, C = meta["nsg"], meta["npc_pad"], meta["C"]
    d_sg, nact, proc, coff, mcols, split = \
        meta["d_sg"], meta["nact"], meta["proc"], meta["coff"], \
        meta["mcols"], meta["split"]
    use_bo = flags["use_bo"]
    use_gn = flags["use_gn"]

    nc = bacc.Bacc("TRN2", target_bir_lowering=False, debug=False)

    e_d = nc.dram_tensor("e_slab", [C * P], F16, kind="ExternalInput").ap()
    w0x_d = nc.dram_tensor("W0x", [P, P], F16, kind="ExternalInput").ap()
    w0m_d = nc.dram_tensor("W0m", [P, P], F16, kind="ExternalInput").ap()
    wh0_d = nc.dram_tensor("Wh0", [P, P], F16, kind="ExternalInput").ap()
    wh1_d = nc.dram_tensor("Wh1", [P, P], F16, kind="ExternalInput").ap()
    wo_d = nc.dram_tensor("Wo", [P, P], F16, kind="ExternalInput").ap()
    b0_d = nc.dram_tensor("b0", [P, 1], F32, kind="ExternalInput").ap()
    bh0_d = nc.dram_tensor("bh0", [P, 1], F32, kind="ExternalInput").ap()
    bh1_d = nc.dram_tensor("bh1", [P, 1], F32, kind="ExternalInput").ap()
    i_d = nc.dram_tensor("I16", [P, P], F16, kind="ExternalInput").ap()
    if use_bo:
        bo_d = nc.dram_tensor("bo_b", [P, SGN], F32,
                              kind="ExternalInput").ap()
    if use_gn:
        gnw_d = nc.dram_tensor("gnw_b", [P, SGN], F32,
                               kind="ExternalInput").ap()
        gnb_d = nc.dram_tensor("gnb_b", [P, SGN], F32,
                               kind="ExternalInput").ap()
    out_d = nc.dram_tensor("out", [P, npc_pad], F16, kind="ExternalOutput").ap()

    with tile.TileContext(nc) as tc:
        with (
            tc.tile_pool(name="const", bufs=1) as cpool,
            tc.tile_pool(name="slab", bufs=6) as spool,
            tc.tile_pool(name="act", bufs=2) as apool,
            tc.tile_pool(name="gn", bufs=2) as gpool,
            tc.tile_pool(name="ot", bufs=2) as opool,
            tc.tile_pool(name="stat", bufs=2) as tpool,
            tc.tile_pool(name="pmsg", bufs=3, space="PSUM") as pmsg,
            tc.tile_pool(name="pmlp", bufs=2, space="PSUM") as pmlp,
            tc.tile_pool(name="pout", bufs=2, space="PSUM") as pout,
            tc.tile_pool(name="pxr", bufs=1, space="PSUM") as pxr,
        ):
            # output batches: OUT_BATCH supergroups each; the last few write
            # per-supergroup so the final drain after the last GN is short.
            sizes = []
            r = nsg
            while r > OUT_BATCH + 3:
                sizes.append(OUT_BATCH)
                r -= OUT_BATCH
            if r >= 4:
                sizes.extend([r - 3, 1, 1, 1])
            else:
                sizes.extend([1] * r)
            batches = []
            bs0 = 0
            for sz in sizes:
                batches.append((bs0, sz))
                bs0 += sz
            bat_idx = {}
            for bi, (bs0, sz) in enumerate(batches):
                for b in range(bs0, bs0 + sz):
                    bat_idx[b] = bi

            # --- PE warm-up: lift the HAM clock gate before real work ---
            # The memset needs no DMA, so the dummy matmuls start as soon as
            # the sequencers boot (~3.5 us) and the PE is at 2.4 GHz by the
            # time the first slab chunk lands.
            wsb = cpool.tile([P, P], F16)
            nc.gpsimd.memset(wsb[:], 0.0625)
            wps = pmsg.tile([P, SGN], F32, tag="msg")
            for _ in range(N_WARM):
                nc.tensor.matmul(wps[:, 0:P], lhsT=wsb[:], rhs=wsb[:],
                                 start=True, stop=True)

            # a 1-column priming DMA eats part of the multi-us cold-start
            # latency of the DMA path while the sequencers still boot.
            prime = cpool.tile([P, 1], F16)
            nc.sync.dma_start(prime[:], i_d[:, 0:1])
            # ident rides first on the sync ring (strict FIFO ahead of the
            # slab stream) so the first message matmuls are never gated on
            # the const ring.
            ident = cpool.tile([P, P], F16)
            nc.sync.dma_start(ident[:], i_d[:])
            eps_t = cpool.tile([P, 1], F32)
            nc.vector.memset(eps_t[:], EPS)

            w0x = cpool.tile([P, P], F16)
            nc.scalar.dma_start(w0x[:], w0x_d[:])
            w0m = cpool.tile([P, P], F16)
            nc.scalar.dma_start(w0m[:], w0m_d[:])
            wh0 = cpool.tile([P, P], F16)
            nc.scalar.dma_start(wh0[:], wh0_d[:])
            wh1 = cpool.tile([P, P], F16)
            nc.scalar.dma_start(wh1[:], wh1_d[:])
            wo = cpool.tile([P, P], F16)
            nc.scalar.dma_start(wo[:], wo_d[:])
            b0 = cpool.tile([P, 1], F32)
            nc.scalar.dma_start(b0[:], b0_d[:])
            bh0 = cpool.tile([P, 1], F32)
            nc.scalar.dma_start(bh0[:], bh0_d[:])
            bh1 = cpool.tile([P, 1], F32)
            nc.scalar.dma_start(bh1[:], bh1_d[:])
            if use_bo:
                bo_b = cpool.tile([P, SGN], F32)
                nc.scalar.dma_start(bo_b[:], bo_d[:])
            if use_gn:
                gnw_b = cpool.tile([P, SGN], F32)
                nc.scalar.dma_start(gnw_b[:], gnw_d[:])
                gnb_b = cpool.tile([P, SGN], F32)
                nc.scalar.dma_start(gnb_b[:], gnb_d[:])

            otp = None
            ob0 = 0
            for b, s in enumerate(proc):
                d = int(d_sg[s])
                na = nact[s]
                mc = int(mcols[b])
                c0 = int(coff[b])
                bi = bat_idx[b]
                ob0, obn = batches[bi]
                slab = spool.tile([P, mc + SGN], F16, tag="slab")
                bounds = split[b]
                for j in range(len(bounds) - 1):
                    lo, hi = bounds[j], bounds[j + 1]
                    nc.sync.dma_start(
                        slab[:, lo:hi],
                        e_d[(c0 + lo) * P:(c0 + hi) * P]
                        .rearrange("(p w) -> p w", p=P))

                # xT is copied out of the slab tile promptly (scalar engine)
                # so the slab buffer recycles as soon as the msg matmuls
                # finish -- late readers must not gate the DMA stream.
                xtc = apool.tile([P, SGN], F16, tag="xt", bufs=3)
                nc.scalar.copy(xtc[:], slab[:, mc:mc + SGN])
                xt = xtc[:]

                # ---- message segment-sum: msgT[feat, node] ----
                # The TensorEngine alone would be the co-bottleneck (1 col/
                # cycle), so adjacent slot PAIRS are pre-combined by fp16
                # adds on the otherwise-idle GpSimd engine: slot 2i's
                # columns absorb slot 2i+1's (both index the same node
                # prefix), then one pass-through matmul per pair feeds the
                # PSUM accumulator.  Narrow taper slots (<= TAPER_W cols)
                # chain-add into the last wide slot the same way.
                d_pe = max(1, sum(1 for k in range(d) if int(na[k]) > TAPER_W))
                starts_k = np.concatenate([[0], np.cumsum(na)]).astype(int)
                # NOTE: offloading wide-slot pair-sums to GpSimd/DVE was
                # tried and is fabric-infeasible: the SBUF read-modify-write
                # traffic (~72 MB) saturates the SBUF AXI ports and stretches
                # every engine.  Only the tiny taper slots leave the PE.
                pe_slots = list(range(d_pe))
                base = int(starts_k[pe_slots[-1]])
                for k in range(d_pe, d):
                    n = int(na[k])
                    nc.gpsimd.tensor_tensor(
                        slab[:, base:base + n],
                        slab[:, base:base + n],
                        slab[:, starts_k[k]:starts_k[k] + n],
                        op=ALU.add)
                msg_ps = pmsg.tile([P, SGN], F32, tag="msg")
                bounds = split[b]
                ci = 1
                for i, k in enumerate(pe_slots):
                    n = int(na[k])
                    # During the DMA ramp (first two supergroups) the PE
                    # waits 1-3us per chunk; a burst of dummy matmuls into
                    # the warm-up bank fills each wait so the PE_HAM clock
                    # gate never drops the PE to 1.2 GHz.  (The warm-up
                    # bank is free until sg2 recycles the msg ring.)
                    if b < 2 and ci < len(bounds) - 1 \
                            and int(starts_k[k]) >= bounds[ci]:
                        ci += 1
                        for _ in range(10):
                            nc.tensor.matmul(wps[:, 0:P], lhsT=wsb[:],
                                             rhs=wsb[:], start=True,
                                             stop=True)
                    nc.tensor.matmul(
                        msg_ps[:, 0:n],
                        lhsT=ident[:],
                        rhs=slab[:, starts_k[k]:starts_k[k] + n],
                        start=(i == 0),
                        stop=(i == len(pe_slots) - 1),
                    )
                msg_s = apool.tile([P, SGN], F16, tag="msg_s")
                nc.scalar.copy(msg_s[:], msg_ps[:])

                # ---- MLP (feature-major, fp16 in / f32 accum) ----

                h_ps = pmlp.tile([P, SGN], F32, tag="mlp")
                nc.tensor.matmul(h_ps[:], lhsT=w0x[:], rhs=xt,
                                 start=True, stop=False)
                nc.tensor.matmul(h_ps[:], lhsT=w0m[:], rhs=msg_s[:],
                                 start=False, stop=True)
                h1 = apool.tile([P, SGN], F16, tag="h")
                nc.scalar.activation(h1[:], h_ps[:], AF.Relu,
                                     bias=b0[:, 0:1])

                h_ps2 = pmlp.tile([P, SGN], F32, tag="mlp")
                nc.tensor.matmul(h_ps2[:], lhsT=wh0[:], rhs=h1[:],
                                 start=True, stop=True)
                h2 = apool.tile([P, SGN], F16, tag="h")
                nc.scalar.activation(h2[:], h_ps2[:], AF.Relu,
                                     bias=bh0[:, 0:1])

                h_ps3 = pmlp.tile([P, SGN], F32, tag="mlp")
                nc.tensor.matmul(h_ps3[:], lhsT=wh1[:], rhs=h2[:],
                                 start=True, stop=True)
                h3 = apool.tile([P, SGN], F16, tag="h")
                nc.scalar.activation(h3[:], h_ps3[:], AF.Relu,
                                     bias=bh1[:, 0:1])

                # ---- output layer, node-major out[node, ch] ----
                o_ps = pout.tile([P, SGN], F32, tag="o")
                for gi in range(NG):
                    nc.tensor.matmul(
                        o_ps[:, gi * P:(gi + 1) * P],
                        lhsT=h3[:, gi * P:(gi + 1) * P],
                        rhs=wo[:],
                        start=True, stop=True,
                    )
                # residual x, node-major, via PE transposes of the xT copy
                # (saves a second x copy from HBM); padded to a full PSUM
                # bank, after the out-layer matmuls so it never delays the
                # msg/MLP chain.
                xr_ps = pxr.tile([P, SGN], F16, tag="xr",
                                 padded_shape=[P, 2 * SGN])
                for gi in range(NG):
                    nc.tensor.transpose(
                        xr_ps[:, gi * P:(gi + 1) * P],
                        xtc[:, gi * P:(gi + 1) * P],
                        ident[:])
                xr = xr_ps[:]

                # ---- GroupNorm(1, C) + residual ----
                # o is copied off PSUM immediately (scalar engine) so the
                # PSUM bank recycles without waiting for the whole GN chain
                if use_bo:
                    basis = gpool.tile([P, SGN], F32, tag="basis")
                    nc.vector.tensor_add(basis[:], o_ps[:], bo_b[:])
                    bsrc = basis
                else:
                    ob_s = gpool.tile([P, SGN], F16, tag="ob")
                    nc.scalar.copy(ob_s[:], o_ps[:])
                    bsrc = ob_s
                b3 = bsrc[:].rearrange("p (g c) -> p g c", c=P)
                # per-group mean/var in one DVE pass (exact for equal
                # even/odd halves, biased var as in the reference)
                mv6 = tpool.tile([P, NG * 6], F32, tag="mv6")
                mv6v = mv6[:].rearrange("p (g s) -> p g s", s=6)
                mv = tpool.tile([P, NG * 2], F32, tag="mv")
                mvv = mv[:].rearrange("p (g s) -> p g s", s=2)
                for gi in range(NG):
                    nc.vector.bn_stats(mv6v[:, gi, :],
                                       bsrc[:, gi * P:(gi + 1) * P])
                    nc.vector.bn_aggr(mvv[:, gi, :], mv6v[:, gi, :])
                sd = tpool.tile([P, NG], F32, tag="sd")
                nc.scalar.activation(sd[:], mvv[:, :, 1], AF.Sqrt,
                                     bias=eps_t[:, 0:1])
                rinv = tpool.tile([P, NG], F32, tag="rinv")
                nc.vector.reciprocal(rinv[:], sd[:])
                musc = tpool.tile([P, NG], F32, tag="musc")
                nc.vector.tensor_tensor(musc[:], mvv[:, :, 0], rinv[:],
                                        op=ALU.mult)

                if b == ob0:
                    otp = opool.tile([P, obn * SGN], F16, tag="ot")
                # out = o*rinv + (x - mu*rinv): one broadcast-subtract for
                # the x term, then a fused (o*rinv)+xn per group.
                musc_b = musc[:, :, None].broadcast_to([P, NG, P])
                xn = gpool.tile([P, SGN], F32, tag="xn")
                nc.vector.tensor_tensor(
                    xn[:].rearrange("p (g c) -> p g c", c=P),
                    xr.rearrange("p (g c) -> p g c", c=P), musc_b,
                    op=ALU.subtract)
                off = (b - ob0) * SGN
                if use_gn:
                    # full affine: out = (o - mu)*rinv*gnw + gnb + x
                    rinv_b = rinv[:, :, None].broadcast_to([P, NG, P])
                    t1 = gpool.tile([P, SGN], F32, tag="t1")
                    nc.vector.tensor_tensor(
                        t1[:].rearrange("p (g c) -> p g c", c=P), b3, rinv_b,
                        op=ALU.mult)
                    nc.vector.tensor_tensor(t1[:], t1[:], gnw_b[:],
                                            op=ALU.mult)
                    gw = gpool.tile([P, SGN], F32, tag="gw")
                    nc.vector.tensor_tensor(
                        gw[:].rearrange("p (g c) -> p g c", c=P),
                        gnw_b[:].rearrange("p (g c) -> p g c", c=P), musc_b,
                        op=ALU.mult)
                    nc.vector.tensor_tensor(
                        xn[:].rearrange("p (g c) -> p g c", c=P),
                        xr.rearrange("p (g c) -> p g c", c=P),
                        gw[:].rearrange("p (g c) -> p g c", c=P),
                        op=ALU.subtract)
                    nc.vector.tensor_add(xn[:], xn[:], gnb_b[:])
                    nc.vector.tensor_add(otp[:, off:off + SGN], t1[:], xn[:])
                else:
                    for gi in range(NG):
                        nc.vector.scalar_tensor_tensor(
                            otp[:, off + gi * P:off + (gi + 1) * P],
                            bsrc[:, gi * P:(gi + 1) * P],
                            rinv[:, gi:gi + 1],
                            xn[:, gi * P:(gi + 1) * P],
                            op0=ALU.mult, op1=ALU.add)
                if b == ob0 + obn - 1:
                    nc.gpsimd.dma_start(
                        out_d[:, ob0 * SGN:(ob0 + obn) * SGN], otp[:])

    return nc


# --------------------------------------------------------------------------
# Entry point
# --------------------------------------------------------------------------

def _run(inputs, trace=False):
    x = np.asarray(inputs["x"], np.float32)
    e = np.asarray(inputs["e"], np.float32)
    edge_index = np.asarray(inputs["edge_index"])
    W0 = np.asarray(inputs["W0"], np.float32)
    b0 = np.asarray(inputs["b0"], np.float32)
    Wh = np.asarray(inputs["Wh"], np.float32)
    bh = np.asarray(inputs["bh"], np.float32)
    Wo = np.asarray(inputs["Wo"], np.float32)
    bo = np.asarray(inputs["bo"], np.float32)
    gn_w = np.asarray(inputs["gn_w"], np.float32)
    gn_b = np.asarray(inputs["gn_b"], np.float32)

    import time as _time
    _t0 = _time.monotonic()
    in_maps, meta = _host_prep(x, e, edge_index)
    print(f"[kernel] host prep {_time.monotonic()-_t0:.1f}s", flush=True)

    flags = dict(use_bo=bool(np.any(bo != 0.0)),
                 use_gn=bool(np.any(gn_w != 1.0) or np.any(gn_b != 0.0)))

    consts = dict(
        W0x=np.ascontiguousarray(W0[:P], np.float16),
        W0m=np.ascontiguousarray(W0[P:], np.float16),
        Wh0=np.ascontiguousarray(Wh[0], np.float16),
        Wh1=np.ascontiguousarray(Wh[1], np.float16),
        Wo=np.ascontiguousarray(Wo, np.float16),
        b0=b0.reshape(P, 1).copy(),
        bh0=bh[0].reshape(P, 1).copy(),
        bh1=bh[1].reshape(P, 1).copy(),
        I16=np.eye(P, dtype=np.float16),
    )
    if flags["use_bo"]:
        consts["bo_b"] = np.tile(bo[None, :], (P, NG)).astype(np.float32)
    if flags["use_gn"]:
        consts["gnw_b"] = np.tile(gn_w[None, :], (P, NG)).astype(np.float32)
        consts["gnb_b"] = np.tile(gn_b[None, :], (P, NG)).astype(np.float32)

    _t0 = _time.monotonic()
    nc = _build_program(meta, flags)
    print(f"[kernel] build {_time.monotonic()-_t0:.1f}s", flush=True)
    _t0 = _time.monotonic()
    nc.compile()
    print(f"[kernel] bacc compile {_time.monotonic()-_t0:.1f}s", flush=True)
    _t0 = _time.monotonic()

    full_maps = []
    for c in range(N_CORES):
        m = dict(in_maps[c])
        m.update(consts)
        full_maps.append(m)

    res = run_bass_kernel_spmd(nc, full_maps, list(range(N_CORES)),
                               trace=trace)
    print(f"[kernel] spmd run {_time.monotonic()-_t0:.1f}s", flush=True)

    n_nodes = x.shape[0]
    out = np.empty((n_nodes, P), np.float32)
    nsg, proc = meta["nsg"], meta["proc"]
    for c, cinfo in enumerate(meta["cores"]):
        oc = np.asarray(res.results[c]["out"]).astype(np.float32)
        # [128 node-in-grp, b, g, ch] -> node-major [b, g, node, ch]
        on = oc.reshape(P, nsg, NG, P).transpose(1, 2, 0, 3)
        perm = cinfo["order"].reshape(nsg, SGN)[proc].reshape(-1)
        valid = perm < cinfo["n_real"]
        out[cinfo["lo"] + perm[valid]] = on.reshape(nsg * SGN, P)[valid]
    return out, res


def kernel(**inputs):
    out, _ = _run(inputs, trace=False)
    return out
